# revision 22
# baseline (speedup 1.0000x reference)
"""Trainium2 Bass kernel for nn_CrossAttention (B=4, NQ=512, NKV=4096, H=12, D=64).

Sharding: 8 cores = 4 batches x 2 head-groups (6 heads each). Each core computes
its (batch, head-group) slice of cross-attention and a partial output projection
(contribution of its 384 attn channels to all 768 output channels). Host sums
the two head-group partials per batch and adds bproj.

Key structure (cost model charges a matmul by its output free size only):
  - attn@V runs "flipped": out[q(128 part), d+1(65 free)] accumulated over kt,
    with a ones column in V giving the softmax denominator in col 64. This
    uses all 128 output partitions (vs 65 in the naive orientation) and makes
    normalization a per-partition scalar multiply.
  - The normalized [q, 2*64] tile is transposed back to [ac, q] with the DMA
    xbar (dma_start_transpose), not the PE.
  - Output projection runs as out[q, oc] with Wproj as the natural rhs.
  - exp runs on Activation (~100us total) while PE (~131us) is kept fed by
    interleaving K/V projection matmuls into the attention kt loops.
Engines: PE matmuls; Act exp; DVE rope muls/adds + norms + psum copies;
GpSimd perm DMAs + V copies; SP input/transpose/output DMAs.
"""

import numpy as np
import ml_dtypes

import concourse.bass as bass
from concourse import bacc
import concourse.mybir as mybir
import concourse.tile as tile
from concourse.bass_utils import run_bass_kernel_spmd

BF16 = ml_dtypes.bfloat16

B, NQ, NKV = 4, 512, 4096
LATENT = 768
H, D = 12, 64
G = 2                  # head groups (cores per batch)
HPG = H // G           # heads per group = 6
DG = HPG * D           # 384 attn channels per group
P = 128
CSUB = LATENT // P     # 6 contraction subtiles
NKT = NKV // P         # 32 k-tiles
NCH = NKV // 512       # 8 512-col data chunks
PAIRS = HPG // 2       # 3 head pairs
QB = NQ // P           # 4 q blocks

FP32 = mybir.dt.float32
BF16_DT = mybir.dt.bfloat16
AOP = mybir.AluOpType
EXP = mybir.ActivationFunctionType.Exp


def _build_program():
    nc = bacc.Bacc()

    def din(name, shape):
        return nc.dram_tensor(name, shape, BF16_DT, kind="ExternalInput")

    latentT = din("latentT", [LATENT, NQ])
    dataT = din("dataT", [LATENT, NKV])
    wq = din("wq", [LATENT, DG])        # pre-scaled by D^-0.5
    wk = din("wk", [LATENT, DG])
    wv = din("wv", [LATENT, DG])
    wproj = din("wproj", [DG, LATENT])
    ropeq = din("ropeq", [P, 2, NQ])    # [128, (cos|sin), n]; 64 rows x2, sin sign-folded
    ropek = din("ropek", [P, 2, NKV])
    ident = din("ident", [P, P])
    out_d = nc.dram_tensor("out", [NQ, LATENT], BF16_DT, kind="ExternalOutput")

    lat_v = latentT.rearrange("(o p) q -> p o q", p=P)
    data_v = dataT.rearrange("(o p) k -> p o k", p=P)
    wq_v = wq.rearrange("(o p) n -> p o n", p=P)
    wk_v = wk.rearrange("(o p) n -> p o n", p=P)
    wv_v = wv.rearrange("(o p) n -> p o n", p=P)
    wproj_v = wproj.rearrange("(o p) n -> p o n", p=P)   # [128, 3, 768]

    PHASE_MARKS.clear()

    def mark(label):
        PHASE_MARKS.append((label, int(nc.get_next_instruction_name()[2:])))

    with tile.TileContext(nc) as tc:
        with (
            tc.tile_pool(name="singles", bufs=1) as singles,
            tc.tile_pool(name="ropep", bufs=2) as ropep,
            tc.tile_pool(name="ep", bufs=3) as ep,
            tc.tile_pool(name="np_pool", bufs=2) as np_pool,
            tc.tile_pool(name="pp", bufs=2, space="PSUM") as pp,
            tc.tile_pool(name="pss", bufs=2, space="PSUM") as pss,
            tc.tile_pool(name="psa", bufs=2, space="PSUM") as psa,
        ):
            # ---- resident SBUF + input DMAs in need order (SP stream) ------
            lat_sb = singles.tile([P, CSUB, NQ], BF16_DT)
            wq_sb = singles.tile([P, CSUB, DG], BF16_DT)
            nc.sync.dma_start(lat_sb[:, 0:3, :], lat_v[:, 0:3, :])
            nc.sync.dma_start(wq_sb[:, 0:3, :], wq_v[:, 0:3, :])
            nc.sync.dma_start(lat_sb[:, 3:6, :], lat_v[:, 3:6, :])
            nc.sync.dma_start(wq_sb[:, 3:6, :], wq_v[:, 3:6, :])
            ropeq_sb = singles.tile([P, 2, NQ], BF16_DT)
            nc.sync.dma_start(ropeq_sb, ropeq[:])
            cosq_sb = ropeq_sb[:, 0, :]
            sinq_sb = ropeq_sb[:, 1, :]
            wk_sb = singles.tile([P, CSUB, DG], BF16_DT)
            nc.sync.dma_start(wk_sb, wk_v)

            data_sb = singles.tile([P, CSUB, NKV], BF16_DT)
            ropek_sb = singles.tile([P, 2, NKV], BF16_DT)
            cosk_sb = ropek_sb[:, 0, :]
            sink_sb = ropek_sb[:, 1, :]

            def dma_data(c):
                sl = slice(c * 512, (c + 1) * 512)
                nc.sync.dma_start(data_sb[:, :, sl], data_v[:, :, sl])

            def dma_rope_k(q):
                sl = slice(q * 1024, (q + 1) * 1024)
                nc.sync.dma_start(ropek_sb[:, :, sl], ropek[:, :, sl])

            dma_data(0)
            dma_data(1)
            wv_sb = singles.tile([P, CSUB, DG], BF16_DT)
            nc.sync.dma_start(wv_sb, wv_v)
            dma_rope_k(0)   # cosk/sink cols 0:1024  (rope quarter 0)
            dma_data(2)
            dma_data(3)
            dma_rope_k(1)
            dma_data(4)
            dma_data(5)
            dma_rope_k(2)
            dma_data(6)
            dma_data(7)
            dma_rope_k(3)
            wproj_sb = singles.tile([P, PAIRS, LATENT], BF16_DT)
            nc.sync.dma_start(wproj_sb, wproj_v)
            ident_sb = singles.tile([P, P], BF16_DT)
            nc.sync.dma_start(ident_sb, ident[:])

            qt_sb = singles.tile([P, PAIRS, NQ], BF16_DT)      # roped Q^T
            kt_sb = [
                singles.tile([P, NKV], BF16_DT, name=f"kt{j}")
                for j in range(PAIRS)
            ]
            cat_sb = [
                singles.tile([P, NQ], BF16_DT, name=f"cat{j}")
                for j in range(PAIRS)
            ]
            v_sb = singles.tile([P, NKT, HPG, D + 1], BF16_DT)
            nc.gpsimd.memset(v_sb[:, :, :, D : D + 1], 1.0)

            # ---- helpers ---------------------------------------------------
            def perm_dma(dst, src, eng=None):
                """dst = src with 32-row halves swapped within each 64-row
                block (the rot-half partition shuffle). eng=nc.scalar uses the
                Activation HWDGE (fast, for the pre-phase while Act is idle);
                default GpSimd SWDGE keeps Act free for exp mid-flight."""
                eng = eng or nc.gpsimd
                for blk in range(2):
                    b0 = blk * 64
                    eng.dma_start(dst[b0 : b0 + 32, :], src[b0 + 32 : b0 + 64, :])
                    eng.dma_start(dst[b0 + 32 : b0 + 64, :], src[b0 : b0 + 32, :])

            # ---- Q projection + rope ---------------------------------------
            qraw = singles.tile([P, PAIRS * NQ], BF16_DT)
            for j in range(PAIRS):
                ps = pp.tile([P, NQ], FP32, tag="pp", name="ps_q")
                for cs in range(CSUB):
                    nc.tensor.matmul(
                        ps,
                        lhsT=wq_sb[:, cs, j * P : (j + 1) * P],
                        rhs=lat_sb[:, cs, :],
                        start=(cs == 0),
                        stop=(cs == CSUB - 1),
                    )
                nc.vector.tensor_copy(qraw[:, j * NQ : (j + 1) * NQ], ps)
            qperm = singles.tile([P, PAIRS * NQ], BF16_DT)
            perm_dma(qperm, qraw, eng=nc.scalar)
            for j in range(PAIRS):
                sl = slice(j * NQ, (j + 1) * NQ)
                nc.vector.tensor_tensor(qraw[:, sl], qraw[:, sl], cosq_sb, AOP.mult)
                nc.vector.tensor_tensor(qperm[:, sl], qperm[:, sl], sinq_sb, AOP.mult)
                nc.vector.tensor_tensor(qt_sb[:, j, :], qraw[:, sl], qperm[:, sl], AOP.add)

            # ---- K projection (per 512-col chunk) + rope (per 1024 quarter)
            kraw = {}

            def kp_chunk(j, ch, copy_eng="dve"):
                sl = slice(ch * 512, (ch + 1) * 512)
                ps = pp.tile([P, 512], FP32, tag="pp", name="ps_k")
                for cs in range(CSUB):
                    nc.tensor.matmul(
                        ps,
                        lhsT=wk_sb[:, cs, j * P : (j + 1) * P],
                        rhs=data_sb[:, cs, sl],
                        start=(cs == 0),
                        stop=(cs == CSUB - 1),
                    )
                quarter = ch // 2
                if ch % 2 == 0:
                    # bufs=6: kraw lifetimes overlap out of rotation order
                    # across pairs (chunks land early, ropes late).
                    kraw[(j, quarter)] = ropep.tile(
                        [P, 1024], BF16_DT, tag="kraw", bufs=6, name=f"kraw{j}_{quarter}"
                    )
                c2 = ch % 2
                dst = kraw[(j, quarter)][:, c2 * 512 : (c2 + 1) * 512]
                if copy_eng == "act":
                    nc.scalar.copy(dst, ps)
                else:
                    nc.vector.tensor_copy(dst, ps)

            def kp_rope(j, quarter, dma_eng=None, mul_eng=None):
                """rope for kt_sb[j] cols [1024q, 1024(q+1)). mul_eng=nc.gpsimd
                moves the combine off DVE (used at phase ends where DVE
                backlog would delay the norms)."""
                mul = mul_eng or nc.vector
                raw = kraw.pop((j, quarter))
                perm = ropep.tile([P, 1024], BF16_DT, tag="kperm", name=f"kperm{j}_{quarter}")
                perm_dma(perm, raw, eng=dma_eng)
                sl = slice(quarter * 1024, (quarter + 1) * 1024)
                mul.tensor_tensor(raw, raw, cosk_sb[:, sl], AOP.mult)
                mul.tensor_tensor(perm, perm, sink_sb[:, sl], AOP.mult)
                mul.tensor_tensor(kt_sb[j][:, sl], raw, perm, AOP.add)

            # ---- V projection for head pair pj, one k-tile -----------------
            # copy_eng: "act" while Activation has slack (pre/att0), else DVE
            def vp(pj, kt, copy_eng="dve"):
                ps = pp.tile([P, P], FP32, tag="pp", name="ps_v")
                for cs in range(CSUB):
                    nc.tensor.matmul(
                        ps,
                        lhsT=data_sb[:, cs, kt * P : (kt + 1) * P],
                        rhs=wv_sb[:, cs, pj * P : (pj + 1) * P],
                        start=(cs == 0),
                        stop=(cs == CSUB - 1),
                    )
                dst = v_sb[:, kt, 2 * pj : 2 * pj + 2, 0:D]
                src = ps.rearrange("p (h d) -> p h d", h=2)
                if copy_eng == "act":
                    nc.scalar.copy(dst, src)
                else:
                    nc.vector.tensor_copy(dst, src)

            # ---- attention: scores+exp for kt, attn@V one iter behind ------
            po = {}
            e_tiles = {}

            def att_scores(j, kt):
                ps_s = pss.tile([P, 2 * NQ], FP32, tag="ss", name="ps_s")
                nc.tensor.matmul(
                    ps_s[:, 0:NQ],
                    lhsT=kt_sb[j][0:64, kt * P : (kt + 1) * P],
                    rhs=qt_sb[0:64, j, :],
                    start=True,
                    stop=True,
                )
                nc.tensor.matmul(
                    ps_s[:, NQ : 2 * NQ],
                    lhsT=kt_sb[j][64:128, kt * P : (kt + 1) * P],
                    rhs=qt_sb[64:128, j, :],
                    start=True,
                    stop=True,
                )
                e = ep.tile([P, 2 * NQ], BF16_DT, tag="e", name="e_pair")
                nc.scalar.activation(e, ps_s, EXP)
                e_tiles[(j, kt)] = e

            def att_av(j, kt):
                e = e_tiles.pop((j, kt))
                if kt == 0:
                    po[(j, 0)] = psa.tile([P, QB, D + 1], FP32, tag="av", name=f"poA{j}")
                    po[(j, 1)] = psa.tile([P, QB, D + 1], FP32, tag="av", name=f"poB{j}")
                for h01 in range(2):
                    for qb in range(QB):
                        # start=True resets has_written for the WHOLE bank, so
                        # only the first region per bank may set it; the other
                        # regions overwrite on their first write (has_written
                        # cleared) and accumulate afterwards.
                        nc.tensor.matmul(
                            po[(j, h01)][:, qb, :],
                            lhsT=e[:, h01 * NQ + qb * P : h01 * NQ + (qb + 1) * P],
                            rhs=v_sb[:, kt, 2 * j + h01, :],
                            start=(kt == 0 and qb == 0),
                            stop=(kt == NKT - 1),
                            skip_group_check=True,
                        )

            def att_phase(j, fillers_of_kt, at_iter1=None):
                """Software-pipelined kt loop: PE order per iter is
                scores(kt) -> fillers -> attnV(kt-1), so exp(kt-1) has a full
                iteration of PE work to hide behind. at_iter1 runs extra
                emissions (previous pair's norms) at the top of iteration 1 —
                after iter 0's fillers so their DVE work isn't queued behind
                the norms' wait on the previous pair's last attn@V."""
                for kt in range(NKT):
                    if kt % 8 == 0:
                        mark(f"att{j}_kt{kt}")
                    if kt == 1 and at_iter1 is not None:
                        at_iter1()
                    att_scores(j, kt)
                    for f in fillers_of_kt(kt):
                        f()
                    if kt > 0:
                        att_av(j, kt - 1)
                att_av(j, NKT - 1)
                mark(f"att{j}_end")

            # ---- normalization + transpose back to [ac, q] -----------------
            def norm_qb(j, qb, pe_tr=False):
                # pe_tr (tail): normalize on Act + transpose on the (idle) PE
                # for minimum latency; otherwise DVE + DMA-xbar transpose.
                rcp = np_pool.tile([P, 2], FP32, tag="rcp", bufs=4, name="rcp")
                nrm = np_pool.tile([P, P], BF16_DT, tag="nrm", bufs=4, name="nrm")
                for h01 in range(2):
                    nc.vector.reciprocal(rcp[:, h01 : h01 + 1], po[(j, h01)][:, qb, D : D + 1])
                    if pe_tr:
                        nc.scalar.activation(
                            nrm[:, h01 * D : (h01 + 1) * D],
                            po[(j, h01)][:, qb, 0:D],
                            mybir.ActivationFunctionType.Copy,
                            scale=rcp[:, h01 : h01 + 1],
                        )
                    else:
                        nc.vector.tensor_scalar(
                            nrm[:, h01 * D : (h01 + 1) * D],
                            po[(j, h01)][:, qb, 0:D],
                            rcp[:, h01 : h01 + 1],
                            None,
                            AOP.mult,
                        )
                if pe_tr:
                    pt = pp.tile([P, P], BF16_DT, tag="pp", name="pt")
                    nc.tensor.transpose(pt, nrm, ident_sb)
                    nc.vector.tensor_copy(cat_sb[j][:, qb * P : (qb + 1) * P], pt)
                else:
                    nc.sync.dma_start_transpose(cat_sb[j][:, qb * P : (qb + 1) * P], nrm)

            # ---- incremental output projection: pair j's contribution to
            # out[qb block, half] accumulated into an fp32 SBUF accumulator,
            # so only pair 2's single matmul chain sits in the tail ----------
            out_acc = singles.tile([P, QB, 2, DG], FP32)
            ob_tiles = {}

            def outproj_partial(j, qb, half):
                ps = pp.tile([P, DG], FP32, tag="pp", name="ps_o")
                nc.tensor.matmul(
                    ps,
                    lhsT=cat_sb[j][:, qb * P : (qb + 1) * P],
                    rhs=wproj_sb[:, j, half * DG : (half + 1) * DG],
                    start=True,
                    stop=True,
                )
                acc = out_acc[:, qb, half, :]
                if j == 0:
                    nc.vector.tensor_copy(acc, ps)
                elif j == 1:
                    nc.vector.tensor_tensor(acc, acc, ps, AOP.add)
                else:
                    # final pair: add into a bf16 staging tile covering two q
                    # blocks; ship both in one DMA when the 4th quarter lands
                    grp = qb // 2
                    if grp not in ob_tiles:
                        ob_tiles[grp] = np_pool.tile(
                            [P, 2, 2, DG], BF16_DT, tag="ob", name="ob"
                        )
                    ob = ob_tiles[grp]
                    nc.vector.tensor_tensor(ob[:, qb % 2, half, :], acc, ps, AOP.add)
                    if qb % 2 == 1 and half == 1:
                        out_ap = out_d[:]
                        dst = bass.AP(
                            tensor=out_ap.tensor,
                            offset=out_ap.offset + grp * 2 * P * LATENT,
                            ap=[[LATENT, P], [P * LATENT, 2], [1, LATENT]],
                        )
                        src_ap = bass.AP(
                            tensor=ob.tensor,
                            offset=ob.offset,
                            ap=[list(a) for a in ob.ap[:1]]
                            + [[2 * DG, 2], [1, 2 * DG]],
                        )
                        nc.sync.dma_start(dst, src_ap)

            # ================ schedule ======================================
            # pre-phase: Q proj/rope; chunks 0-1 of K for all pairs; V kts
            # 0-3 for all pairs; rope quarter 0 of all pairs. Everything else
            # spreads uniformly over the three attention phases, each phase
            # filling with its own pair's remaining K chunks/ropes + V tiles
            # (+ the previous pair's output-projection partials).
            mark("pre")
            # PE warm-up: spin the tensor engine on junk during the initial
            # input-DMA wait so the p-state ramp (3us to full clock) finishes
            # before real work arrives.
            warm = ropep.tile([P, 640], BF16_DT, tag="warm", bufs=1, name="warm")
            nc.vector.memset(warm, 0.0)
            for w in range(22):
                ps_w = pss.tile([P, 512], FP32, tag="ss", name="ps_w")
                nc.tensor.matmul(
                    ps_w, lhsT=warm[:, 0:P], rhs=warm[:, P : P + 512],
                    start=True, stop=True,
                )
            kp_chunk(0, 0, "act")
            kp_chunk(0, 1, "act")
            for k in range(4):
                vp(0, k, "act")
            kp_rope(0, 0, dma_eng=nc.sync)   # SP-inline: lowest latency here
            kp_chunk(1, 0, "act")
            kp_chunk(1, 1, "act")
            for k in range(4):
                vp(1, k, "act")
            kp_chunk(2, 0, "act")
            kp_chunk(2, 1, "act")
            for k in range(4):
                vp(2, k, "act")
            kp_rope(1, 0)                    # GpSimd perms: latency-tolerant
            kp_rope(2, 0)

            # per-phase filler: own-pair K chunks (arrival-aware for pair 0),
            # own ropes, own V kts 4-31 with 4-iter lookahead
            def make_fill(j, kp_iters, rope_iters, extra=None):
                def fill(kt):
                    fillers = []
                    if kt in kp_iters:
                        fillers.append(lambda c=kp_iters[kt]: kp_chunk(j, c))
                    if kt in rope_iters:
                        q = rope_iters[kt]
                        fillers.append(lambda q=q: kp_rope(j, q))
                    if 0 <= kt < 28:
                        fillers.append(lambda k=kt + 4: vp(j, k))
                    if extra is not None:
                        fillers.extend(extra(kt))
                    return fillers
                return fill

            # att(0): kp placement tracks data-chunk arrival (d45 ~24us,
            # d67 ~31us at ~1.2us/iter from t~20)
            att_phase(0, make_fill(
                0,
                {0: 2, 1: 3, 4: 4, 5: 5, 9: 6, 10: 7},
                {2: 1, 6: 2, 11: 3},
            ))

            def norms(j):
                def go():
                    for qb in range(QB):
                        norm_qb(j, qb)
                return go

            def extra1(kt):
                out = []
                if 16 <= kt < 24:
                    qb, h = (kt - 16) // 2, kt % 2
                    out.append(lambda qb=qb, h=h: outproj_partial(0, qb, h))
                return out

            att_phase(1, make_fill(
                1,
                {0: 2, 1: 3, 2: 4, 3: 5, 4: 6, 5: 7},
                {2: 1, 6: 2, 8: 3},
                extra1,
            ), at_iter1=norms(0))

            def extra2(kt):
                out = []
                if 16 <= kt < 24:
                    qb, h = (kt - 16) // 2, kt % 2
                    out.append(lambda qb=qb, h=h: outproj_partial(1, qb, h))
                return out

            att_phase(2, make_fill(
                2,
                {0: 2, 1: 3, 2: 4, 3: 5, 4: 6, 5: 7},
                {2: 1, 6: 2, 8: 3},
                extra2,
            ), at_iter1=norms(1))

            # tail: normalize pair 2 (PE transpose), outproj + output DMAs
            mark("tail")
            for qb in range(QB):
                norm_qb(2, qb, pe_tr=True)
                outproj_partial(2, qb, 0)
                outproj_partial(2, qb, 1)

    nc.finalize()
    return nc


PHASE_MARKS = []


_NC_CACHE = None


def _get_program():
    global _NC_CACHE
    if _NC_CACHE is None:
        _NC_CACHE = _build_program()
    return _NC_CACHE


def _host_inputs(latent, data, rope_q, rope_k, Wq, bq, Wkv, bkv, Wproj, bproj):
    assert not np.any(bq) and not np.any(bkv), "nonzero qkv biases unsupported"
    scale = D ** -0.5
    sign = np.concatenate([-np.ones(32, np.float32), np.ones(32, np.float32)])

    def rep(x):  # [64, n] -> [128, n], two head-copies
        return np.concatenate([x, x], axis=0).astype(BF16)

    sin_q, cos_q = rope_q[:, :D].T, rope_q[:, D:].T      # [64, 512]
    sin_k, cos_k = rope_k[:, :D].T, rope_k[:, D:].T      # [64, 4096]
    ropeq_r = np.stack([rep(cos_q), rep(sign[:, None] * sin_q)], axis=1)
    ropek_r = np.stack([rep(cos_k), rep(sign[:, None] * sin_k)], axis=1)

    in_maps = []
    for c in range(8):
        b, g = c // 2, c % 2
        sl = slice(g * DG, (g + 1) * DG)
        in_maps.append({
            "latentT": np.ascontiguousarray(latent[b].T).astype(BF16),
            "dataT": np.ascontiguousarray(data[b].T).astype(BF16),
            "wq": (Wq[:, sl] * scale).astype(BF16),
            "wk": Wkv[:, g * DG : (g + 1) * DG].astype(BF16),
            "wv": Wkv[:, LATENT + g * DG : LATENT + (g + 1) * DG].astype(BF16),
            "wproj": Wproj[sl, :].astype(BF16),
            "ropeq": ropeq_r, "ropek": ropek_r,
            "ident": np.eye(P, dtype=np.float32).astype(BF16),
        })
    return in_maps


def kernel(latent, data, rope_q, rope_k, Wq, bq, Wkv, bkv, Wproj, bproj,
           _trace=False):
    nc = _get_program()
    in_maps = _host_inputs(latent, data, rope_q, rope_k, Wq, bq, Wkv, bkv,
                           Wproj, bproj)
    res = run_bass_kernel_spmd(nc, in_maps, core_ids=list(range(8)),
                               trace=_trace)
    out = np.empty((B, NQ, LATENT), np.float32)
    for b in range(B):
        acc = (res.results[2 * b]["out"].astype(np.float32)
               + res.results[2 * b + 1]["out"].astype(np.float32))
        out[b] = acc + bproj[None, :]
    kernel.last_results = res
    return out


# revision 23
# speedup vs baseline: 1.0819x; 1.0819x over previous
"""Trainium2 Bass kernel for nn_CrossAttention (B=4, NQ=512, NKV=4096, H=12, D=64).

Sharding: 8 cores = 4 batches x 2 head-groups (6 heads each). Each core computes
its (batch, head-group) slice of cross-attention and a partial output projection
(contribution of its 384 attn channels to all 768 output channels). Host sums
the two head-group partials per batch and adds bproj.

Key structure (cost model charges a matmul by its output free size only):
  - attn@V runs "flipped": out[q(128 part), d+1(65 free)] accumulated over kt,
    with a ones column in V giving the softmax denominator in col 64. This
    uses all 128 output partitions (vs 65 in the naive orientation) and makes
    normalization a per-partition scalar multiply.
  - The normalized [q, 2*64] tile is transposed back to [ac, q] with the DMA
    xbar (dma_start_transpose), not the PE.
  - Output projection runs as out[q, oc] with Wproj as the natural rhs.
  - exp runs on Activation (~100us total) while PE (~131us) is kept fed by
    interleaving K/V projection matmuls into the attention kt loops.
Engines: PE matmuls; Act exp; DVE rope muls/adds + norms + psum copies;
GpSimd perm DMAs + V copies; SP input/transpose/output DMAs.
"""

import numpy as np
import ml_dtypes

import concourse.bass as bass
from concourse import bacc
import concourse.mybir as mybir
import concourse.tile as tile
from concourse.bass_utils import run_bass_kernel_spmd

BF16 = ml_dtypes.bfloat16

B, NQ, NKV = 4, 512, 4096
LATENT = 768
H, D = 12, 64
G = 2                  # head groups (cores per batch)
HPG = H // G           # heads per group = 6
DG = HPG * D           # 384 attn channels per group
P = 128
CSUB = LATENT // P     # 6 contraction subtiles
NKT = NKV // P         # 32 k-tiles
NCH = NKV // 512       # 8 512-col data chunks
PAIRS = HPG // 2       # 3 head pairs
QB = NQ // P           # 4 q blocks

FP32 = mybir.dt.float32
BF16_DT = mybir.dt.bfloat16
AOP = mybir.AluOpType
EXP = mybir.ActivationFunctionType.Exp


def _build_program():
    nc = bacc.Bacc()

    def din(name, shape):
        return nc.dram_tensor(name, shape, BF16_DT, kind="ExternalInput")

    latentT = din("latentT", [LATENT, NQ])
    dataT = din("dataT", [LATENT, NKV])
    wq = din("wq", [LATENT, DG])        # pre-scaled by D^-0.5
    wk = din("wk", [LATENT, DG])
    wv = din("wv", [LATENT, DG])
    wproj = din("wproj", [DG, LATENT])
    ropeq = din("ropeq", [P, 2, NQ])    # [128, (cos|sin), n]; 64 rows x2, sin sign-folded
    ropek = din("ropek", [P, 2, NKV])
    ident = din("ident", [P, P])
    out_d = nc.dram_tensor("out", [NQ, LATENT], BF16_DT, kind="ExternalOutput")

    lat_v = latentT.rearrange("(o p) q -> p o q", p=P)
    data_v = dataT.rearrange("(o p) k -> p o k", p=P)
    wq_v = wq.rearrange("(o p) n -> p o n", p=P)
    wk_v = wk.rearrange("(o p) n -> p o n", p=P)
    wv_v = wv.rearrange("(o p) n -> p o n", p=P)
    wproj_v = wproj.rearrange("(o p) n -> p o n", p=P)   # [128, 3, 768]

    PHASE_MARKS.clear()

    def mark(label):
        PHASE_MARKS.append((label, int(nc.get_next_instruction_name()[2:])))

    with tile.TileContext(nc) as tc:
        with (
            tc.tile_pool(name="singles", bufs=1) as singles,
            tc.tile_pool(name="ropep", bufs=2) as ropep,
            tc.tile_pool(name="ep", bufs=3) as ep,
            tc.tile_pool(name="np_pool", bufs=2) as np_pool,
            tc.tile_pool(name="pp", bufs=2, space="PSUM") as pp,
            tc.tile_pool(name="pss", bufs=2, space="PSUM") as pss,
            tc.tile_pool(name="psa", bufs=2, space="PSUM") as psa,
        ):
            # ---- resident SBUF + input DMAs in need order (SP stream) ------
            lat_sb = singles.tile([P, CSUB, NQ], BF16_DT)
            wq_sb = singles.tile([P, CSUB, DG], BF16_DT)
            nc.sync.dma_start(lat_sb[:, 0:3, :], lat_v[:, 0:3, :])
            nc.sync.dma_start(wq_sb[:, 0:3, :], wq_v[:, 0:3, :])
            nc.sync.dma_start(lat_sb[:, 3:6, :], lat_v[:, 3:6, :])
            nc.sync.dma_start(wq_sb[:, 3:6, :], wq_v[:, 3:6, :])
            ropeq_sb = singles.tile([P, 2, NQ], BF16_DT)
            nc.sync.dma_start(ropeq_sb, ropeq[:])
            cosq_sb = ropeq_sb[:, 0, :]
            sinq_sb = ropeq_sb[:, 1, :]
            wk_sb = singles.tile([P, CSUB, DG], BF16_DT)
            nc.sync.dma_start(wk_sb, wk_v)

            data_sb = singles.tile([P, CSUB, NKV], BF16_DT)
            ropek_sb = singles.tile([P, 2, NKV], BF16_DT)
            cosk_sb = ropek_sb[:, 0, :]
            sink_sb = ropek_sb[:, 1, :]

            def dma_data(c):
                sl = slice(c * 512, (c + 1) * 512)
                nc.sync.dma_start(data_sb[:, :, sl], data_v[:, :, sl])

            def dma_rope_k(q):
                sl = slice(q * 1024, (q + 1) * 1024)
                nc.sync.dma_start(ropek_sb[:, :, sl], ropek[:, :, sl])

            dma_data(0)
            dma_data(1)
            wv_sb = singles.tile([P, CSUB, DG], BF16_DT)
            nc.sync.dma_start(wv_sb, wv_v)
            dma_rope_k(0)   # cosk/sink cols 0:1024  (rope quarter 0)
            dma_data(2)
            dma_data(3)
            dma_rope_k(1)
            dma_data(4)
            dma_data(5)
            dma_rope_k(2)
            dma_data(6)
            dma_data(7)
            dma_rope_k(3)
            wproj_sb = singles.tile([P, PAIRS, LATENT], BF16_DT)
            nc.sync.dma_start(wproj_sb, wproj_v)
            ident_sb = singles.tile([P, P], BF16_DT)
            nc.sync.dma_start(ident_sb, ident[:])

            qt_sb = singles.tile([P, PAIRS, NQ], BF16_DT)      # roped Q^T
            kt_sb = [
                singles.tile([P, NKV], BF16_DT, name=f"kt{j}")
                for j in range(PAIRS)
            ]
            cat_sb = [
                singles.tile([P, NQ], BF16_DT, name=f"cat{j}")
                for j in range(PAIRS)
            ]
            v_sb = singles.tile([P, NKT, HPG, D + 1], BF16_DT)
            nc.gpsimd.memset(v_sb[:, :, :, D : D + 1], 1.0)

            # ---- helpers ---------------------------------------------------
            def perm_dma(dst, src, eng=None):
                """dst = src with 32-row halves swapped within each 64-row
                block (the rot-half partition shuffle). eng=nc.scalar uses the
                Activation HWDGE (fast, for the pre-phase while Act is idle);
                default GpSimd SWDGE keeps Act free for exp mid-flight."""
                eng = eng or nc.gpsimd
                for blk in range(2):
                    b0 = blk * 64
                    eng.dma_start(dst[b0 : b0 + 32, :], src[b0 + 32 : b0 + 64, :])
                    eng.dma_start(dst[b0 + 32 : b0 + 64, :], src[b0 : b0 + 32, :])

            # ---- Q projection + rope ---------------------------------------
            qraw = singles.tile([P, PAIRS * NQ], BF16_DT)
            for j in range(PAIRS):
                ps = pp.tile([P, NQ], FP32, tag="pp", name="ps_q")
                for cs in range(CSUB):
                    nc.tensor.matmul(
                        ps,
                        lhsT=wq_sb[:, cs, j * P : (j + 1) * P],
                        rhs=lat_sb[:, cs, :],
                        start=(cs == 0),
                        stop=(cs == CSUB - 1),
                    )
                nc.vector.tensor_copy(qraw[:, j * NQ : (j + 1) * NQ], ps)
            qperm = singles.tile([P, PAIRS * NQ], BF16_DT)
            perm_dma(qperm, qraw, eng=nc.scalar)
            for j in range(PAIRS):
                sl = slice(j * NQ, (j + 1) * NQ)
                nc.vector.tensor_tensor(qraw[:, sl], qraw[:, sl], cosq_sb, AOP.mult)
                nc.vector.tensor_tensor(qperm[:, sl], qperm[:, sl], sinq_sb, AOP.mult)
                nc.vector.tensor_tensor(qt_sb[:, j, :], qraw[:, sl], qperm[:, sl], AOP.add)

            # ---- K projection (per 512-col chunk) + rope (per 1024 quarter)
            kraw = {}

            def kp_chunk(j, ch, copy_eng="dve"):
                sl = slice(ch * 512, (ch + 1) * 512)
                ps = pp.tile([P, 512], FP32, tag="pp", name="ps_k")
                for cs in range(CSUB):
                    nc.tensor.matmul(
                        ps,
                        lhsT=wk_sb[:, cs, j * P : (j + 1) * P],
                        rhs=data_sb[:, cs, sl],
                        start=(cs == 0),
                        stop=(cs == CSUB - 1),
                    )
                quarter = ch // 2
                if ch % 2 == 0:
                    # bufs=6: kraw lifetimes overlap out of rotation order
                    # across pairs (chunks land early, ropes late).
                    kraw[(j, quarter)] = ropep.tile(
                        [P, 1024], BF16_DT, tag="kraw", bufs=6, name=f"kraw{j}_{quarter}"
                    )
                c2 = ch % 2
                dst = kraw[(j, quarter)][:, c2 * 512 : (c2 + 1) * 512]
                if copy_eng == "act":
                    nc.scalar.copy(dst, ps)
                else:
                    nc.vector.tensor_copy(dst, ps)

            def kp_rope(j, quarter, dma_eng=None, mul_eng=None):
                """rope for kt_sb[j] cols [1024q, 1024(q+1)). mul_eng=nc.gpsimd
                moves the combine off DVE (used at phase ends where DVE
                backlog would delay the norms)."""
                mul = mul_eng or nc.vector
                raw = kraw.pop((j, quarter))
                perm = ropep.tile([P, 1024], BF16_DT, tag="kperm", name=f"kperm{j}_{quarter}")
                perm_dma(perm, raw, eng=dma_eng)
                sl = slice(quarter * 1024, (quarter + 1) * 1024)
                mul.tensor_tensor(raw, raw, cosk_sb[:, sl], AOP.mult)
                mul.tensor_tensor(perm, perm, sink_sb[:, sl], AOP.mult)
                mul.tensor_tensor(kt_sb[j][:, sl], raw, perm, AOP.add)

            # ---- V projection for head pair pj, one k-tile -----------------
            # copy_eng: "act" while Activation has slack (pre/att0), else DVE
            def vp(pj, kt, copy_eng="dve"):
                ps = pp.tile([P, P], FP32, tag="pp", name="ps_v")
                for cs in range(CSUB):
                    nc.tensor.matmul(
                        ps,
                        lhsT=data_sb[:, cs, kt * P : (kt + 1) * P],
                        rhs=wv_sb[:, cs, pj * P : (pj + 1) * P],
                        start=(cs == 0),
                        stop=(cs == CSUB - 1),
                    )
                dst = v_sb[:, kt, 2 * pj : 2 * pj + 2, 0:D]
                src = ps.rearrange("p (h d) -> p h d", h=2)
                if copy_eng == "act":
                    nc.scalar.copy(dst, src)
                else:
                    nc.vector.tensor_copy(dst, src)

            # ---- attention: scores+exp for kt, attn@V one iter behind ------
            po = {}
            e_tiles = {}

            def att_scores(j, kt):
                ps_s = pss.tile([P, 2 * NQ], FP32, tag="ss", name="ps_s")
                nc.tensor.matmul(
                    ps_s[:, 0:NQ],
                    lhsT=kt_sb[j][0:64, kt * P : (kt + 1) * P],
                    rhs=qt_sb[0:64, j, :],
                    start=True,
                    stop=True,
                )
                nc.tensor.matmul(
                    ps_s[:, NQ : 2 * NQ],
                    lhsT=kt_sb[j][64:128, kt * P : (kt + 1) * P],
                    rhs=qt_sb[64:128, j, :],
                    start=True,
                    stop=True,
                )
                e = ep.tile([P, 2 * NQ], BF16_DT, tag="e", name="e_pair")
                nc.scalar.activation(e, ps_s, EXP)
                e_tiles[(j, kt)] = e

            def att_av(j, kt):
                e = e_tiles.pop((j, kt))
                if kt == 0:
                    po[(j, 0)] = psa.tile([P, QB, D + 1], FP32, tag="av", name=f"poA{j}")
                    po[(j, 1)] = psa.tile([P, QB, D + 1], FP32, tag="av", name=f"poB{j}")
                for h01 in range(2):
                    for qb in range(QB):
                        # start=True resets has_written for the WHOLE bank, so
                        # only the first region per bank may set it; the other
                        # regions overwrite on their first write (has_written
                        # cleared) and accumulate afterwards.
                        nc.tensor.matmul(
                            po[(j, h01)][:, qb, :],
                            lhsT=e[:, h01 * NQ + qb * P : h01 * NQ + (qb + 1) * P],
                            rhs=v_sb[:, kt, 2 * j + h01, :],
                            start=(kt == 0 and qb == 0),
                            stop=(kt == NKT - 1),
                            skip_group_check=True,
                        )

            def att_phase(j, fillers_of_kt, at_iter1=None):
                """Software-pipelined kt loop: PE order per iter is
                scores(kt) -> fillers -> attnV(kt-1), so exp(kt-1) has a full
                iteration of PE work to hide behind. at_iter1 runs extra
                emissions (previous pair's norms) at the top of iteration 1 —
                after iter 0's fillers so their DVE work isn't queued behind
                the norms' wait on the previous pair's last attn@V."""
                for kt in range(NKT):
                    if kt % 8 == 0:
                        mark(f"att{j}_kt{kt}")
                    if kt == 1 and at_iter1 is not None:
                        at_iter1()
                    att_scores(j, kt)
                    for f in fillers_of_kt(kt):
                        f()
                    if kt > 0:
                        att_av(j, kt - 1)
                att_av(j, NKT - 1)
                mark(f"att{j}_end")

            # ---- normalization + transpose back to [ac, q] -----------------
            def norm_qb(j, qb, pe_tr=False):
                # pe_tr (tail): normalize on Act + transpose on the (idle) PE
                # for minimum latency; otherwise DVE + DMA-xbar transpose.
                rcp = np_pool.tile([P, 2], FP32, tag="rcp", bufs=4, name="rcp")
                nrm = np_pool.tile([P, P], BF16_DT, tag="nrm", bufs=4, name="nrm")
                for h01 in range(2):
                    nc.vector.reciprocal(rcp[:, h01 : h01 + 1], po[(j, h01)][:, qb, D : D + 1])
                    if pe_tr:
                        nc.scalar.activation(
                            nrm[:, h01 * D : (h01 + 1) * D],
                            po[(j, h01)][:, qb, 0:D],
                            mybir.ActivationFunctionType.Copy,
                            scale=rcp[:, h01 : h01 + 1],
                        )
                    else:
                        nc.vector.tensor_scalar(
                            nrm[:, h01 * D : (h01 + 1) * D],
                            po[(j, h01)][:, qb, 0:D],
                            rcp[:, h01 : h01 + 1],
                            None,
                            AOP.mult,
                        )
                if pe_tr:
                    pt = pp.tile([P, P], BF16_DT, tag="pp", name="pt")
                    nc.tensor.transpose(pt, nrm, ident_sb)
                    nc.vector.tensor_copy(cat_sb[j][:, qb * P : (qb + 1) * P], pt)
                else:
                    nc.sync.dma_start_transpose(cat_sb[j][:, qb * P : (qb + 1) * P], nrm)

            # ---- incremental output projection: pair j's contribution to
            # out[qb block, half] accumulated into an fp32 SBUF accumulator,
            # so only pair 2's single matmul chain sits in the tail ----------
            out_acc = singles.tile([P, QB, 2, DG], FP32)
            ob_tiles = {}

            def outproj_partial(j, qb, half):
                ps = pp.tile([P, DG], FP32, tag="pp", name="ps_o")
                nc.tensor.matmul(
                    ps,
                    lhsT=cat_sb[j][:, qb * P : (qb + 1) * P],
                    rhs=wproj_sb[:, j, half * DG : (half + 1) * DG],
                    start=True,
                    stop=True,
                )
                acc = out_acc[:, qb, half, :]
                if j == 0:
                    nc.vector.tensor_copy(acc, ps)
                elif j == 1:
                    nc.vector.tensor_tensor(acc, acc, ps, AOP.add)
                else:
                    # final pair: add into a bf16 staging tile covering two q
                    # blocks; ship both in one DMA when the 4th quarter lands
                    grp = qb // 2
                    if grp not in ob_tiles:
                        ob_tiles[grp] = np_pool.tile(
                            [P, 2, 2, DG], BF16_DT, tag="ob", name="ob"
                        )
                    ob = ob_tiles[grp]
                    nc.vector.tensor_tensor(ob[:, qb % 2, half, :], acc, ps, AOP.add)
                    if qb % 2 == 1 and half == 1:
                        out_ap = out_d[:]
                        dst = bass.AP(
                            tensor=out_ap.tensor,
                            offset=out_ap.offset + grp * 2 * P * LATENT,
                            ap=[[LATENT, P], [P * LATENT, 2], [1, LATENT]],
                        )
                        src_ap = bass.AP(
                            tensor=ob.tensor,
                            offset=ob.offset,
                            ap=[list(a) for a in ob.ap[:1]]
                            + [[2 * DG, 2], [1, 2 * DG]],
                        )
                        nc.sync.dma_start(dst, src_ap)

            # ================ schedule ======================================
            # pre-phase: Q proj/rope; chunks 0-1 of K for all pairs; V kts
            # 0-3 for all pairs; rope quarter 0 of all pairs. Everything else
            # spreads uniformly over the three attention phases, each phase
            # filling with its own pair's remaining K chunks/ropes + V tiles
            # (+ the previous pair's output-projection partials).
            mark("pre")
            # PE warm-up: spin the tensor engine on junk during the initial
            # input-DMA wait so the p-state ramp (3us to full clock) finishes
            # before real work arrives.
            warm = ropep.tile([P, 640], BF16_DT, tag="warm", bufs=1, name="warm")
            nc.vector.memset(warm, 0.0)
            for w in range(22):
                ps_w = pss.tile([P, 512], FP32, tag="ss", name="ps_w")
                nc.tensor.matmul(
                    ps_w, lhsT=warm[:, 0:P], rhs=warm[:, P : P + 512],
                    start=True, stop=True,
                )
            kp_chunk(0, 0, "act")
            kp_chunk(0, 1, "act")
            for k in range(4):
                vp(0, k, "act")
            kp_rope(0, 0, dma_eng=nc.sync)   # SP-inline: lowest latency here
            kp_chunk(1, 0, "act")
            kp_chunk(1, 1, "act")
            for k in range(4):
                vp(1, k, "act")
            kp_chunk(2, 0, "act")
            kp_chunk(2, 1, "act")
            for k in range(4):
                vp(2, k, "act")
            kp_rope(1, 0)                    # GpSimd perms: latency-tolerant
            kp_rope(2, 0)

            # per-phase filler: own-pair K chunks (arrival-aware for pair 0),
            # own ropes, own V kts 4-31 with 4-iter lookahead
            def make_fill(j, kp_iters, rope_iters, extra=None, rope_dma=None):
                def fill(kt):
                    fillers = []
                    if kt in kp_iters:
                        fillers.append(lambda c=kp_iters[kt]: kp_chunk(j, c))
                    if kt in rope_iters:
                        q = rope_iters[kt]
                        mul = nc.gpsimd if q == 3 else None
                        fillers.append(
                            lambda q=q, m=mul: kp_rope(j, q, dma_eng=rope_dma, mul_eng=m)
                        )
                    if 0 <= kt < 28:
                        fillers.append(lambda k=kt + 4: vp(j, k))
                    if extra is not None:
                        fillers.extend(extra(kt))
                    return fillers
                return fill

            # att(0): kp placement tracks data-chunk arrival (d45 ~24us,
            # d67 ~31us at ~1.2us/iter from t~20)
            att_phase(0, make_fill(
                0,
                {0: 2, 1: 3, 4: 4, 5: 5, 9: 6, 10: 7},
                {2: 1, 6: 2, 11: 3},
            ))

            def norms(j):
                def go():
                    for qb in range(QB):
                        norm_qb(j, qb)
                return go

            def extra1(kt):
                out = []
                if 16 <= kt < 24:
                    qb, h = (kt - 16) // 2, kt % 2
                    out.append(lambda qb=qb, h=h: outproj_partial(0, qb, h))
                return out

            att_phase(1, make_fill(
                1,
                {0: 2, 1: 3, 2: 4, 3: 5, 4: 6, 5: 7},
                {2: 1, 6: 2, 8: 3},
                extra1,
                rope_dma=nc.sync,
            ), at_iter1=norms(0))

            def extra2(kt):
                out = []
                if 16 <= kt < 24:
                    qb, h = (kt - 16) // 2, kt % 2
                    out.append(lambda qb=qb, h=h: outproj_partial(1, qb, h))
                return out

            att_phase(2, make_fill(
                2,
                {0: 2, 1: 3, 2: 4, 3: 5, 4: 6, 5: 7},
                {2: 1, 6: 2, 8: 3},
                extra2,
                rope_dma=nc.sync,
            ), at_iter1=norms(1))

            # tail: normalize pair 2 (PE transpose), outproj + output DMAs
            mark("tail")
            for qb in range(QB):
                norm_qb(2, qb, pe_tr=True)
                outproj_partial(2, qb, 0)
                outproj_partial(2, qb, 1)

    nc.finalize()
    return nc


PHASE_MARKS = []


_NC_CACHE = None


def _get_program():
    global _NC_CACHE
    if _NC_CACHE is None:
        _NC_CACHE = _build_program()
    return _NC_CACHE


def _host_inputs(latent, data, rope_q, rope_k, Wq, bq, Wkv, bkv, Wproj, bproj):
    assert not np.any(bq) and not np.any(bkv), "nonzero qkv biases unsupported"
    scale = D ** -0.5
    sign = np.concatenate([-np.ones(32, np.float32), np.ones(32, np.float32)])

    def rep(x):  # [64, n] -> [128, n], two head-copies
        return np.concatenate([x, x], axis=0).astype(BF16)

    sin_q, cos_q = rope_q[:, :D].T, rope_q[:, D:].T      # [64, 512]
    sin_k, cos_k = rope_k[:, :D].T, rope_k[:, D:].T      # [64, 4096]
    ropeq_r = np.stack([rep(cos_q), rep(sign[:, None] * sin_q)], axis=1)
    ropek_r = np.stack([rep(cos_k), rep(sign[:, None] * sin_k)], axis=1)

    in_maps = []
    for c in range(8):
        b, g = c // 2, c % 2
        sl = slice(g * DG, (g + 1) * DG)
        in_maps.append({
            "latentT": np.ascontiguousarray(latent[b].T).astype(BF16),
            "dataT": np.ascontiguousarray(data[b].T).astype(BF16),
            "wq": (Wq[:, sl] * scale).astype(BF16),
            "wk": Wkv[:, g * DG : (g + 1) * DG].astype(BF16),
            "wv": Wkv[:, LATENT + g * DG : LATENT + (g + 1) * DG].astype(BF16),
            "wproj": Wproj[sl, :].astype(BF16),
            "ropeq": ropeq_r, "ropek": ropek_r,
            "ident": np.eye(P, dtype=np.float32).astype(BF16),
        })
    return in_maps


def kernel(latent, data, rope_q, rope_k, Wq, bq, Wkv, bkv, Wproj, bproj,
           _trace=False):
    nc = _get_program()
    in_maps = _host_inputs(latent, data, rope_q, rope_k, Wq, bq, Wkv, bkv,
                           Wproj, bproj)
    res = run_bass_kernel_spmd(nc, in_maps, core_ids=list(range(8)),
                               trace=_trace)
    out = np.empty((B, NQ, LATENT), np.float32)
    for b in range(B):
        acc = (res.results[2 * b]["out"].astype(np.float32)
               + res.results[2 * b + 1]["out"].astype(np.float32))
        out[b] = acc + bproj[None, :]
    kernel.last_results = res
    return out
